# revision 1
# baseline (speedup 1.0000x reference)
"""GCN encoder (3-layer GraphConvolution + scatter) on 8 TRN2 NeuronCores.

Strategy (dest-sharded message passing):
  - Nodes padded to N_pad = C*BLOCKS*128, dest rows sharded across 8 cores.
  - Per layer: each core computes support = h_shard @ W for its shard
    (dense matmuls), AllGather replicates the support table into each
    core's HBM (Shared scratchpad).
  - Message phase: per 128-row dest block, dma_gather pulls the ~4K source
    rows (edges grouped by dest block, split into lo/hi half-tables since
    gather indices are int16), a one-hot(dest)*val matrix is built on the
    DVE from 1-byte local-dest ids via iota/is_equal, and the TensorEngine
    segment-sums messages into PSUM (K-tile accumulation). Bias is folded
    in as an extra K-tile; ReLU on the ScalarEngine.
  - Layer 3 epilogue scatters rows straight to the padded output via
    indirect DMA (pos_idx), relying on pre-zeroed output buffers.
  - Host only shards/packs inputs and sums the per-core outputs (disjoint).
"""

import math
import os
import sys

import numpy as np

for _p in ("/opt/trn_rl_repo",):
    if _p not in sys.path and os.path.isdir(_p):
        sys.path.insert(0, _p)

import ml_dtypes

import concourse.bass as bass
import concourse.bacc as bacc
import concourse.mybir as mybir
import concourse.tile as tile
from concourse.bass_utils import run_bass_kernel_spmd

P = 128
C = 8
N_LAYERS = 3

F32 = mybir.dt.float32
I16 = mybir.dt.int16
I32 = mybir.dt.int32

# bf16 data path for the gather table / messages / one-hot (accumulation
# stays fp32 in PSUM). Toggle with KERNEL_FP32=1.
BF16 = not os.environ.get("KERNEL_FP32")
GDT = mybir.dt.bfloat16 if BF16 else mybir.dt.float32
NP_GDT = ml_dtypes.bfloat16 if BF16 else np.float32

# Filled in by kernel() for test harnesses to inspect.
LAST_RESULTS = None


class _Cfg:
    def __init__(self, n_real, nfeat, nhid, pad_n, tiles_per_half):
        self.n_real = n_real
        self.nfeat = nfeat
        self.nhid = nhid
        self.pad_n = pad_n
        self.blocks = math.ceil(n_real / (C * P))  # dest blocks per core
        self.shard = self.blocks * P
        self.n_pad = C * self.shard
        self.half = self.n_pad // 2
        assert self.half <= 32767, "gather half-table exceeds int16 range"
        self.T = tiles_per_half

    def key(self):
        return (self.n_real, self.nfeat, self.nhid, self.pad_n, self.blocks, self.T)


def _wrap_idx(idx):
    """[..., n] int16 -> [..., 128, n//16] wrapped over 16 partitions, replicated."""
    shp = idx.shape[:-1]
    n = idx.shape[-1]
    assert n % 16 == 0
    w = idx.reshape(*shp, n // 16, 16)
    w = np.swapaxes(w, -1, -2)  # [..., 16, n//16]
    w = np.broadcast_to(w[..., None, :, :], (*shp, 8, 16, n // 16))
    return np.ascontiguousarray(w).reshape(*shp, 128, n // 16)


def _host_prep(x, motif_emb, adj_rows, adj_cols, adj_vals, pos_idx, pad_n,
               w1, b1, w2, b2, w3, b3):
    n_x, nfeat = x.shape
    n_motif = motif_emb.shape[0]
    n_real = n_x + n_motif
    nhid = w1.shape[1]
    pad_n = int(pad_n)

    rows = np.asarray(adj_rows).astype(np.int64)
    cols = np.asarray(adj_cols).astype(np.int64)
    vals = np.asarray(adj_vals).astype(np.float32)
    pos_idx = np.asarray(pos_idx).astype(np.int64)

    blocks = math.ceil(n_real / (C * P))
    shard = blocks * P
    n_pad = C * shard
    half = n_pad // 2
    nblk = C * blocks

    # Group edges by (dest block, src half); order within a group is free.
    half_flag = (cols >= half).astype(np.int64)
    sel = np.lexsort((half_flag, rows // P))
    r_s = rows[sel]
    c_s = cols[sel]
    v_s = vals[sel]
    h_s = half_flag[sel]
    key = (r_s // P) * 2 + h_s
    grp_starts = np.searchsorted(key, np.arange(nblk * 2))
    grp_ends = np.searchsorted(key, np.arange(nblk * 2) + 1)
    counts = grp_ends - grp_starts
    T = max(1, int(math.ceil(counts.max() / P)))

    cfg = _Cfg(n_real, nfeat, nhid, pad_n, T)
    slots = T * P

    E = len(rows)
    pos_in_grp = np.arange(E) - grp_starts[key]
    slot = key * slots + pos_in_grp
    gidx_flat = np.zeros(nblk * 2 * slots, np.int16)
    val_flat = np.zeros(nblk * 2 * slots, np.float32)
    ld_flat = np.zeros(nblk * 2 * slots, np.float32)
    gidx_flat[slot] = (c_s - h_s * half).astype(np.int16)
    val_flat[slot] = v_s
    ld_flat[slot] = (r_s % P).astype(np.float32)

    # gidx: [C, blocks, 2, 128, 8T] wrapped int16
    gidx = _wrap_idx(gidx_flat.reshape(C, blocks, 2, slots))
    # ld/vals: [C, 128, blocks, 2T]   (edge slot e of tile t -> partition e%128, col t)
    def _edge_layout(a):
        a = a.reshape(C, blocks, 2 * T, P)          # [C, b, t, e]
        return np.ascontiguousarray(a.transpose(0, 3, 1, 2))
    ld = _edge_layout(ld_flat).astype(NP_GDT)
    vv = _edge_layout(val_flat).astype(NP_GDT)

    # h0 padded + per-core transposed shard
    h0 = np.concatenate(
        [np.asarray(x, np.float32), np.asarray(motif_emb, np.float32)], axis=0)
    if n_pad > n_real:
        h0 = np.concatenate([h0, np.zeros((n_pad - n_real, nfeat), np.float32)], 0)
    h0t = np.ascontiguousarray(
        h0.reshape(C, shard, nfeat).transpose(0, 2, 1))  # [C, nfeat, shard]

    # scatter positions [C, 128, blocks] int32 (1<<20 = skip)
    g = np.arange(n_pad).reshape(C, blocks, P)
    pos = np.full((C, blocks, P), 1 << 20, np.int64)
    m = g < n_x
    pos[m] = pos_idx[g[m]]
    pos = np.ascontiguousarray(pos.transpose(0, 2, 1)).astype(np.int32)

    # weights / bias / consts
    ws = [np.asarray(w, np.float32) for w in (w1, w2, w3)][:N_LAYERS]
    biasrow = np.zeros((N_LAYERS, P, nhid), NP_GDT)
    for i, b in enumerate((b1, b2, b3)[:N_LAYERS]):
        biasrow[i, 0, :] = np.asarray(b, np.float32)
    iota = np.tile(np.arange(P, dtype=np.float32), (P, 1)).astype(NP_GDT)
    identity = np.eye(P, dtype=np.float32)
    onesrow = np.zeros((P, P), NP_GDT)
    onesrow[0, :] = 1.0

    in_maps = []
    for c in range(C):
        im = {
            "h0t": h0t[c],
            "gidx": gidx[c],
            "ld": ld[c],
            "vals": vv[c],
            "pos": pos[c],
            "biasrow": biasrow,
            "iota": iota,
            "identity": identity,
            "onesrow": onesrow,
        }
        for i, w in enumerate(ws):
            im[f"w{i}"] = w
        in_maps.append(im)
    return cfg, in_maps


def _build_program(cfg):
    T = cfg.T
    nhid = cfg.nhid
    nfeat = cfg.nfeat
    blocks = cfg.blocks
    K1 = nfeat // P   # k-tiles for layer 1 support
    K2 = nhid // P    # k-tiles for layers 2/3 support
    assert nfeat % P == 0 and nhid % P == 0

    nc = bacc.Bacc("TRN2", target_bir_lowering=False, debug=False, num_devices=C,
                   dynamic_dma_scratch_size=int(os.environ.get("KERNEL_DDS", 16384)))

    h0t_d = nc.dram_tensor("h0t", [nfeat, cfg.shard], F32, kind="ExternalInput")
    gidx_d = nc.dram_tensor("gidx", [blocks, 2, P, 8 * T], I16, kind="ExternalInput")
    ld_d = nc.dram_tensor("ld", [P, blocks, 2 * T], GDT, kind="ExternalInput")
    vals_d = nc.dram_tensor("vals", [P, blocks, 2 * T], GDT, kind="ExternalInput")
    pos_d = nc.dram_tensor("pos", [P, blocks], I32, kind="ExternalInput")
    biasrow_d = nc.dram_tensor("biasrow", [N_LAYERS, P, nhid], GDT, kind="ExternalInput")
    iota_d = nc.dram_tensor("iota", [P, P], GDT, kind="ExternalInput")
    ident_d = nc.dram_tensor("identity", [P, P], F32, kind="ExternalInput")
    ones_d = nc.dram_tensor("onesrow", [P, P], GDT, kind="ExternalInput")
    w_d = [
        nc.dram_tensor(f"w{l}", [nfeat if l == 0 else nhid, nhid], F32,
                       kind="ExternalInput")
        for l in range(N_LAYERS)
    ]
    out_d = nc.dram_tensor("out", [cfg.pad_n, nhid], F32, kind="ExternalOutput")

    with tile.TileContext(nc) as tc:
        with tc.tile_pool(name="const", bufs=1) as cpool, \
             tc.tile_pool(name="edge", bufs=1) as epool, \
             tc.tile_pool(name="gidx", bufs=3) as gpool, \
             tc.tile_pool(name="msgs", bufs=2) as mpool, \
             tc.tile_pool(name="onehot", bufs=2) as opool, \
             tc.tile_pool(name="sup", bufs=3) as spool, \
             tc.tile_pool(name="hsb", bufs=3) as hpool, \
             tc.tile_pool(name="psum_m", bufs=4, space="PSUM") as pmpool, \
             tc.tile_pool(name="psum_s", bufs=2, space="PSUM") as pspool, \
             tc.tile_pool(name="psum_t", bufs=2, space="PSUM") as ptpool, \
             tc.tile_pool(name="dram", bufs=N_LAYERS * int(os.environ.get("KERNEL_REPEAT", 1)), space="DRAM") as dpool:

            # ---- resident constants ----
            iota_t = cpool.tile([P, P], GDT)
            nc.sync.dma_start(iota_t[:], iota_d[:, :])
            ident_t = cpool.tile([P, P], F32)
            nc.sync.dma_start(ident_t[:], ident_d[:, :])
            ones_t = cpool.tile([P, P], GDT)
            nc.sync.dma_start(ones_t[:], ones_d[:, :])
            biasrow_t = cpool.tile([P, N_LAYERS, nhid], GDT)
            nc.sync.dma_start(biasrow_t[:], biasrow_d[:, :, :].transpose([1, 0, 2]))
            w_t = []
            for l in range(N_LAYERS):
                kt = K1 if l == 0 else K2
                wt = cpool.tile([P, kt, nhid], F32, tag=f"w{l}", name=f"wt{l}")
                nc.sync.dma_start(
                    wt[:],
                    w_d[l][:, :].rearrange("(k p) n -> p k n", p=P))
                w_t.append(wt)
            pos_t = cpool.tile([P, blocks], I32)
            nc.sync.dma_start(pos_t[:], pos_d[:, :])
            ld_t = epool.tile([P, blocks, 2 * T], GDT)
            nc.sync.dma_start(ld_t[:], ld_d[:, :, :])
            vals_t = epool.tile([P, blocks, 2 * T], GDT)
            nc.sync.dma_start(vals_t[:], vals_d[:, :, :])

            h_dram = [None] * N_LAYERS  # h after layer l (device layout [shard, nhid])

            reps = int(os.environ.get("KERNEL_REPEAT", 1))
            for rep in range(reps):
             for l in range(N_LAYERS):
                kt = K1 if l == 0 else K2
                # ---------- support phase ----------
                mine = dpool.tile([cfg.shard, nhid], GDT, tag="mine")
                table = dpool.tile([cfg.n_pad, nhid], GDT, tag="table",
                                   addr_space="Shared")
                for b in range(blocks):
                    lhs_t = spool.tile([P, kt, P], F32, tag="lhs")
                    if l == 0:
                        nc.sync.dma_start(
                            lhs_t[:],
                            h0t_d[:, b * P:(b + 1) * P].rearrange(
                                "(k p) n -> p k n", p=P))
                    else:
                        hb = spool.tile([P, nhid], F32, tag="hb")
                        nc.sync.dma_start(
                            hb[:], h_dram[l - 1][b * P:(b + 1) * P, :])
                        for k in range(kt):
                            pt = ptpool.tile([P, P], F32, space="PSUM")
                            nc.tensor.transpose(
                                out=pt[:], in_=hb[:, k * P:(k + 1) * P],
                                identity=ident_t[:])
                            nc.scalar.copy(lhs_t[:, k, :], pt[:])
                    ps = pspool.tile([P, nhid], F32, space="PSUM")
                    for k in range(kt):
                        nc.tensor.matmul(
                            ps[:], lhs_t[:, k, :], w_t[l][:, k, :],
                            start=(k == 0), stop=(k == kt - 1))
                    s_sb = spool.tile([P, nhid], GDT, tag="ssb")
                    nc.scalar.copy(s_sb[:], ps[:])
                    nc.sync.dma_start(mine[b * P:(b + 1) * P, :], s_sb[:])

                nc.gpsimd.collective_compute(
                    "AllGather", mybir.AluOpType.bypass,
                    replica_groups=[list(range(C))],
                    ins=[mine[:].opt()], outs=[table[:].opt()])

                # ---------- message phase ----------
                if l < N_LAYERS - 1:
                    h_dram[l] = dpool.tile([cfg.shard, nhid], F32, tag="h",
                                           name=f"h_l{l}")
                for b in range(blocks):
                    gi = gpool.tile([P, 2, 8 * T], I16)
                    nc.sync.dma_start(
                        gi[:], gidx_d[b, :, :, :].transpose([1, 0, 2]))
                    msgs = mpool.tile([P, 2 * T, nhid], GDT)
                    nc.gpsimd.dma_gather(
                        msgs[:, 0:T, :], table[:cfg.half, :], gi[:, 0, :],
                        T * P, T * P, nhid, single_packet=False)
                    nc.gpsimd.dma_gather(
                        msgs[:, T:2 * T, :], table[cfg.half:, :], gi[:, 1, :],
                        T * P, T * P, nhid, single_packet=False)
                    oh = opool.tile([P, 2 * T, P], GDT)
                    nc.vector.tensor_tensor(
                        out=oh[:],
                        in0=ld_t[:, b, :].to_broadcast([P, 2 * T, P]),
                        in1=iota_t[:].unsqueeze(1).to_broadcast([P, 2 * T, P]),
                        op=mybir.AluOpType.is_equal)
                    nc.vector.tensor_tensor(
                        out=oh[:], in0=oh[:],
                        in1=vals_t[:, b, :].to_broadcast([P, 2 * T, P]),
                        op=mybir.AluOpType.mult)
                    pm = pmpool.tile([P, nhid], F32, space="PSUM")
                    nc.tensor.matmul(
                        pm[:], ones_t[:], biasrow_t[:, l, :],
                        start=True, stop=False)
                    for t in range(2 * T):
                        nc.tensor.matmul(
                            pm[:], oh[:, t, :], msgs[:, t, :],
                            start=False, stop=(t == 2 * T - 1))
                    h_sb = hpool.tile([P, nhid], F32)
                    nc.scalar.activation(
                        h_sb[:], pm[:], mybir.ActivationFunctionType.Relu)
                    if l < N_LAYERS - 1:
                        nc.sync.dma_start(
                            h_dram[l][b * P:(b + 1) * P, :], h_sb[:])
                    else:
                        nc.gpsimd.indirect_dma_start(
                            out=out_d[:, :],
                            out_offset=bass.IndirectOffsetOnAxis(
                                ap=pos_t[:, b:b + 1], axis=0),
                            in_=h_sb[:],
                            in_offset=None,
                            bounds_check=cfg.pad_n - 1,
                            oob_is_err=False)

    nc.compile()
    return nc


_CACHE = {}


def kernel(**inputs):
    global LAST_RESULTS
    cfg, in_maps = _host_prep(**inputs)
    k = cfg.key()
    if k not in _CACHE:
        _CACHE[k] = _build_program(cfg)
    nc = _CACHE[k]
    if os.environ.get("KERNEL_SIM"):
        from concourse.bass_interp import MultiCoreSim
        sim = MultiCoreSim(nc, num_cores=C, require_finite=True,
                           require_nnan=True)
        for c in range(C):
            cs = sim.cores[c]
            for name, arr in in_maps[c].items():
                cs.tensor(name)[:] = arr
            cs.tensor("out")[:] = 0.0
        sim.simulate(check_with_hw=False)
        outs = [np.array(sim.cores[c].tensor("out")) for c in range(C)]
        LAST_RESULTS = None
    else:
        res = None
        last_exc = None
        for attempt in range(3):
            try:
                res = run_bass_kernel_spmd(nc, in_maps, core_ids=list(range(C)))
                break
            except Exception as exc:  # flaky axon worker / wedged device
                last_exc = exc
                print(f"kernel: attempt {attempt} failed: {exc}", file=sys.stderr)
        if res is None:
            raise last_exc
        LAST_RESULTS = res
        outs = [res.results[c]["out"] for c in range(C)]
    out = outs[0].astype(np.float32).copy()
    for c in range(1, C):
        out += outs[c]
    return out


# ---------------------------------------------------------------- self test
def _np_reference(x, motif_emb, adj_rows, adj_cols, adj_vals, pos_idx, pad_n,
                  w1, b1, w2, b2, w3, b3):
    h = np.concatenate([x, motif_emb], 0).astype(np.float64)
    n = h.shape[0]
    for w, b in ((w1, b1), (w2, b2), (w3, b3)):
        sup = h @ w.astype(np.float64)
        msgs = adj_vals[:, None].astype(np.float64) * sup[adj_cols]
        agg = np.zeros((n, w.shape[1]), np.float64)
        np.add.at(agg, adj_rows, msgs)
        h = np.maximum(agg + b, 0.0)
    h = h[: x.shape[0]]
    out = np.zeros((int(pad_n), h.shape[1]), np.float64)
    out[pos_idx] = h
    return out.astype(np.float32)


def _self_test(n_x=2800, n_motif=200, e=96000, nfeat=512, nhid=256, pad_n=4096,
               seed=0):
    rng = np.random.default_rng(seed)
    n = n_x + n_motif
    inputs = dict(
        x=rng.standard_normal((n_x, nfeat), dtype=np.float32),
        motif_emb=rng.standard_normal((n_motif, nfeat), dtype=np.float32),
        adj_rows=rng.integers(0, n, e),
        adj_cols=rng.integers(0, n, e),
        adj_vals=rng.random(e, dtype=np.float32),
        pos_idx=rng.permutation(pad_n)[:n_x],
        pad_n=np.int64(pad_n),
        w1=(rng.random((nfeat, nhid), dtype=np.float32) - 0.5) / np.sqrt(nhid),
        b1=(rng.random(nhid, dtype=np.float32) - 0.5) / np.sqrt(nhid),
        w2=(rng.random((nhid, nhid), dtype=np.float32) - 0.5) / np.sqrt(nhid),
        b2=(rng.random(nhid, dtype=np.float32) - 0.5) / np.sqrt(nhid),
        w3=(rng.random((nhid, nhid), dtype=np.float32) - 0.5) / np.sqrt(nhid),
        b3=(rng.random(nhid, dtype=np.float32) - 0.5) / np.sqrt(nhid),
    )
    expected = _np_reference(**inputs)
    got = kernel(**inputs)
    denom = np.abs(expected).max()
    err = np.abs(got - expected).max() / denom
    print(f"self-test abs-max rel err: {err:.3e}  (denom {denom:.3f})")
    assert err < 2e-3, "self test FAILED"
    print("SELF TEST PASS")


if __name__ == "__main__":
    _self_test()



# revision 2
# speedup vs baseline: 10.6580x; 10.6580x over previous
"""GCN encoder (3-layer GraphConvolution + scatter) on 8 TRN2 NeuronCores.

Strategy (dest-sharded message passing):
  - Nodes padded to N_pad = C*BLOCKS*128, dest rows sharded across 8 cores.
  - Per layer: each core computes support = h_shard @ W for its shard
    (dense bf16 matmuls), AllGather replicates the support table into each
    core's HBM (Shared scratchpad).
  - Message phase: per 128-row dest block, dma_gather pulls the source
    rows (edges grouped by dest block, split into lo/hi half-tables since
    gather indices are int16).  Pad slots carry index -1 and a per-group
    edge count is loaded into a Pool register (num_idxs_reg), so the DMA
    only transfers real edges.  A one-hot(dest)*val matrix is built on the
    DVE from local-dest ids via iota/is_equal, and the TensorEngine
    segment-sums messages into PSUM (K-tile accumulation).  Bias is folded
    in as an extra K-tile; ReLU on the ScalarEngine.
  - Layer 3 writes the per-shard result densely to HBM; the host performs
    the pos_idx scatter into the padded output (allowed: kernel() may
    gather/unshard on host).
  - Host only shards/packs inputs and scatters the per-core outputs.
"""

import math
import os
import sys

import numpy as np

for _p in ("/opt/trn_rl_repo",):
    if _p not in sys.path and os.path.isdir(_p):
        sys.path.insert(0, _p)

import ml_dtypes

import concourse.bass as bass
import concourse.bacc as bacc
import concourse.mybir as mybir
import concourse.tile as tile
from concourse.bass_utils import run_bass_kernel_spmd

P = 128
C = 8
N_LAYERS = 3

F32 = mybir.dt.float32
I16 = mybir.dt.int16
I32 = mybir.dt.int32

# bf16 data path for the gather table / messages / one-hot / support matmuls
# (accumulation stays fp32 in PSUM). Toggle with KERNEL_FP32=1.
BF16 = not os.environ.get("KERNEL_FP32")
GDT = mybir.dt.bfloat16 if BF16 else mybir.dt.float32
NP_GDT = ml_dtypes.bfloat16 if BF16 else np.float32

# Filled in by kernel() for test harnesses to inspect.
LAST_RESULTS = None


class _Cfg:
    def __init__(self, n_real, nfeat, nhid, pad_n, tiles_per_half):
        self.n_real = n_real
        self.nfeat = nfeat
        self.nhid = nhid
        self.pad_n = pad_n
        self.blocks = math.ceil(n_real / (C * P))  # dest blocks per core
        self.shard = self.blocks * P
        self.n_pad = C * self.shard
        self.half = self.n_pad // 2
        assert self.half <= 32767, "gather half-table exceeds int16 range"
        self.T = tiles_per_half

    def key(self):
        return (self.n_real, self.nfeat, self.nhid, self.pad_n, self.blocks, self.T)


def _wrap_idx(idx):
    """[..., n] int16 -> [..., 128, n//16] wrapped over 16 partitions, replicated."""
    shp = idx.shape[:-1]
    n = idx.shape[-1]
    assert n % 16 == 0
    w = idx.reshape(*shp, n // 16, 16)
    w = np.swapaxes(w, -1, -2)  # [..., 16, n//16]
    w = np.broadcast_to(w[..., None, :, :], (*shp, 8, 16, n // 16))
    return np.ascontiguousarray(w).reshape(*shp, 128, n // 16)


def _host_prep(x, motif_emb, adj_rows, adj_cols, adj_vals, pos_idx, pad_n,
               w1, b1, w2, b2, w3, b3):
    n_x, nfeat = x.shape
    n_motif = motif_emb.shape[0]
    n_real = n_x + n_motif
    nhid = w1.shape[1]
    pad_n = int(pad_n)

    rows = np.asarray(adj_rows).astype(np.int64)
    cols = np.asarray(adj_cols).astype(np.int64)
    vals = np.asarray(adj_vals).astype(np.float32)
    pos_idx = np.asarray(pos_idx).astype(np.int64)

    blocks = math.ceil(n_real / (C * P))
    shard = blocks * P
    n_pad = C * shard
    half = n_pad // 2
    nblk = C * blocks

    # Group edges by (dest block, src half); order within a group is free.
    half_flag = (cols >= half).astype(np.int64)
    sel = np.lexsort((half_flag, rows // P))
    r_s = rows[sel]
    c_s = cols[sel]
    v_s = vals[sel]
    h_s = half_flag[sel]
    key = (r_s // P) * 2 + h_s
    grp_starts = np.searchsorted(key, np.arange(nblk * 2))
    grp_ends = np.searchsorted(key, np.arange(nblk * 2) + 1)
    counts = grp_ends - grp_starts
    T = max(1, int(math.ceil(counts.max() / P)))

    cfg = _Cfg(n_real, nfeat, nhid, pad_n, T)
    slots = T * P

    E = len(rows)
    pos_in_grp = np.arange(E) - grp_starts[key]
    slot = key * slots + pos_in_grp
    # pad slots: idx -1 (dma_gather skips trailing negatives), val/ld 0
    gidx_flat = np.full(nblk * 2 * slots, -1, np.int16)
    val_flat = np.zeros(nblk * 2 * slots, np.float32)
    ld_flat = np.zeros(nblk * 2 * slots, np.float32)
    gidx_flat[slot] = (c_s - h_s * half).astype(np.int16)
    val_flat[slot] = v_s
    ld_flat[slot] = (r_s % P).astype(np.float32)

    # a group with zero edges still needs one valid index (the executor
    # requires at least one non-negative idx per gather)
    cnt = counts.copy()
    empty = np.nonzero(cnt == 0)[0]
    if len(empty):
        gidx_flat[empty * slots] = 0
        cnt[empty] = 1

    # gidx: [C, blocks, 2, 128, 8T] wrapped int16
    gidx = _wrap_idx(gidx_flat.reshape(C, blocks, 2, slots))
    # ld/vals: [C, 128, blocks, 2T]   (edge slot e of tile t -> partition e%128, col t)
    def _edge_layout(a):
        a = a.reshape(C, blocks, 2 * T, P)          # [C, b, t, e]
        return np.ascontiguousarray(a.transpose(0, 3, 1, 2))
    ld = _edge_layout(ld_flat).astype(NP_GDT)
    vv = _edge_layout(val_flat).astype(NP_GDT)
    # per-(block, half) real-edge counts, int32 [C, 1, 2*blocks]
    cnt = cnt.reshape(C, blocks * 2).astype(np.int32)[:, None, :]

    # h0 padded + per-core transposed shard (bf16 on the wire)
    h0 = np.concatenate(
        [np.asarray(x, np.float32), np.asarray(motif_emb, np.float32)], axis=0)
    if n_pad > n_real:
        h0 = np.concatenate([h0, np.zeros((n_pad - n_real, nfeat), np.float32)], 0)
    h0t = np.ascontiguousarray(
        h0.reshape(C, shard, nfeat).transpose(0, 2, 1)).astype(NP_GDT)

    # weights / bias / consts
    ws = [np.asarray(w, NP_GDT) for w in (w1, w2, w3)][:N_LAYERS]
    biasrow = np.zeros((N_LAYERS, P, nhid), NP_GDT)
    for i, b in enumerate((b1, b2, b3)[:N_LAYERS]):
        biasrow[i, 0, :] = np.asarray(b, np.float32)
    iota = np.tile(np.arange(P, dtype=np.float32), (P, 1)).astype(NP_GDT)
    identity = np.eye(P, dtype=np.float32)
    onesrow = np.zeros((P, P), NP_GDT)
    onesrow[0, :] = 1.0

    in_maps = []
    for c in range(C):
        im = {
            "h0t": h0t[c],
            "gidx": gidx[c],
            "ld": ld[c],
            "vals": vv[c],
            "cnt": cnt[c],
            "biasrow": biasrow,
            "iota": iota,
            "identity": identity,
            "onesrow": onesrow,
        }
        for i, w in enumerate(ws):
            im[f"w{i}"] = w
        in_maps.append(im)
    return cfg, in_maps


def _build_program(cfg):
    T = cfg.T
    nhid = cfg.nhid
    nfeat = cfg.nfeat
    blocks = cfg.blocks
    K1 = nfeat // P   # k-tiles for layer 1 support
    K2 = nhid // P    # k-tiles for layers 2/3 support
    assert nfeat % P == 0 and nhid % P == 0

    nc = bacc.Bacc("TRN2", target_bir_lowering=False, debug=False, num_devices=C,
                   dynamic_dma_scratch_size=int(os.environ.get("KERNEL_DDS", 16384)))

    h0t_d = nc.dram_tensor("h0t", [nfeat, cfg.shard], GDT, kind="ExternalInput")
    gidx_d = nc.dram_tensor("gidx", [blocks, 2, P, 8 * T], I16, kind="ExternalInput")
    ld_d = nc.dram_tensor("ld", [P, blocks, 2 * T], GDT, kind="ExternalInput")
    vals_d = nc.dram_tensor("vals", [P, blocks, 2 * T], GDT, kind="ExternalInput")
    cnt_d = nc.dram_tensor("cnt", [1, blocks * 2], I32, kind="ExternalInput")
    biasrow_d = nc.dram_tensor("biasrow", [N_LAYERS, P, nhid], GDT, kind="ExternalInput")
    iota_d = nc.dram_tensor("iota", [P, P], GDT, kind="ExternalInput")
    ident_d = nc.dram_tensor("identity", [P, P], F32, kind="ExternalInput")
    ones_d = nc.dram_tensor("onesrow", [P, P], GDT, kind="ExternalInput")
    w_d = [
        nc.dram_tensor(f"w{l}", [nfeat if l == 0 else nhid, nhid], GDT,
                       kind="ExternalInput")
        for l in range(N_LAYERS)
    ]
    out_d = nc.dram_tensor("out", [cfg.shard, nhid], F32, kind="ExternalOutput")

    with tile.TileContext(nc) as tc:
        with tc.tile_pool(name="const", bufs=1) as cpool, \
             tc.tile_pool(name="edge", bufs=1) as epool, \
             tc.tile_pool(name="gidx", bufs=3) as gpool, \
             tc.tile_pool(name="onehot", bufs=2) as opool, \
             tc.tile_pool(name="sup", bufs=3) as spool, \
             tc.tile_pool(name="hsb", bufs=3) as hpool, \
             tc.tile_pool(name="psum_m", bufs=4, space="PSUM") as pmpool, \
             tc.tile_pool(name="psum_s", bufs=2, space="PSUM") as pspool, \
             tc.tile_pool(name="psum_t", bufs=2, space="PSUM") as ptpool, \
             tc.tile_pool(name="dram", bufs=N_LAYERS * int(os.environ.get("KERNEL_REPEAT", 1)), space="DRAM") as dpool:

            # ---- resident constants ----
            iota_t = cpool.tile([P, P], GDT)
            nc.sync.dma_start(iota_t[:], iota_d[:, :])
            ident_t = cpool.tile([P, P], F32)
            nc.sync.dma_start(ident_t[:], ident_d[:, :])
            ones_t = cpool.tile([P, P], GDT)
            nc.sync.dma_start(ones_t[:], ones_d[:, :])
            biasrow_t = cpool.tile([P, N_LAYERS, nhid], GDT)
            nc.sync.dma_start(biasrow_t[:], biasrow_d[:, :, :].transpose([1, 0, 2]))
            w_t = []
            for l in range(N_LAYERS):
                kt = K1 if l == 0 else K2
                wt = cpool.tile([P, kt, nhid], GDT, tag=f"w{l}", name=f"wt{l}")
                nc.sync.dma_start(
                    wt[:],
                    w_d[l][:, :].rearrange("(k p) n -> p k n", p=P))
                w_t.append(wt)
            cnt_t = cpool.tile([1, blocks * 2], I32)
            nc.sync.dma_start(cnt_t[:], cnt_d[:, :])
            ld_t = epool.tile([P, blocks, 2 * T], GDT)
            nc.sync.dma_start(ld_t[:], ld_d[:, :, :])
            vals_t = epool.tile([P, blocks, 2 * T], GDT)
            nc.sync.dma_start(vals_t[:], vals_d[:, :, :])

            # manually double-buffered message tiles, zeroed once so pad
            # slots (skipped by the gather) never hold non-finite data
            msgs_bufs = [epool.tile([P, 2 * T, nhid], GDT, name=f"msgs{i}")
                         for i in range(2)]
            for mb in msgs_bufs:
                nc.vector.memset(mb[:], 0.0)

            cnt_regs = [nc.gpsimd.alloc_register(f"cntreg{i}") for i in range(2)]

            h_dram = [None] * N_LAYERS  # h after layer l (device layout [shard, nhid])

            reps = int(os.environ.get("KERNEL_REPEAT", 1))
            for rep in range(reps):
             for l in range(N_LAYERS):
                kt = K1 if l == 0 else K2
                # ---------- support phase ----------
                mine = dpool.tile([cfg.shard, nhid], GDT, tag="mine")
                table = dpool.tile([cfg.n_pad, nhid], GDT, tag="table",
                                   addr_space="Shared")
                for b in range(blocks):
                    lhs_t = spool.tile([P, kt, P], GDT, tag="lhs")
                    if l == 0:
                        nc.sync.dma_start(
                            lhs_t[:],
                            h0t_d[:, b * P:(b + 1) * P].rearrange(
                                "(k p) n -> p k n", p=P))
                    else:
                        hb = spool.tile([P, nhid], F32, tag="hb")
                        nc.sync.dma_start(
                            hb[:], h_dram[l - 1][b * P:(b + 1) * P, :])
                        for k in range(kt):
                            pt = ptpool.tile([P, P], F32, space="PSUM")
                            nc.tensor.transpose(
                                out=pt[:], in_=hb[:, k * P:(k + 1) * P],
                                identity=ident_t[:])
                            nc.scalar.copy(lhs_t[:, k, :], pt[:])
                    ps = pspool.tile([P, nhid], F32, space="PSUM")
                    for k in range(kt):
                        nc.tensor.matmul(
                            ps[:], lhs_t[:, k, :], w_t[l][:, k, :],
                            start=(k == 0), stop=(k == kt - 1))
                    s_sb = spool.tile([P, nhid], GDT, tag="ssb")
                    nc.scalar.copy(s_sb[:], ps[:])
                    nc.sync.dma_start(mine[b * P:(b + 1) * P, :], s_sb[:])

                nc.gpsimd.collective_compute(
                    "AllGather", mybir.AluOpType.bypass,
                    replica_groups=[list(range(C))],
                    ins=[mine[:].opt()], outs=[table[:].opt()])

                # ---------- message phase ----------
                if l < N_LAYERS - 1:
                    h_dram[l] = dpool.tile([cfg.shard, nhid], F32, tag="h",
                                           name=f"h_l{l}")
                for b in range(blocks):
                    gi = gpool.tile([P, 2, 8 * T], I16)
                    nc.sync.dma_start(
                        gi[:], gidx_d[b, :, :, :].transpose([1, 0, 2]))
                    msgs = msgs_bufs[b % 2]
                    nc.gpsimd.reg_load(cnt_regs[0], cnt_t[0:1, 2 * b:2 * b + 1])
                    nc.gpsimd.dma_gather(
                        msgs[:, 0:T, :], table[:cfg.half, :], gi[:, 0, :],
                        T * P, cnt_regs[0], nhid, single_packet=False)
                    nc.gpsimd.reg_load(cnt_regs[1], cnt_t[0:1, 2 * b + 1:2 * b + 2])
                    nc.gpsimd.dma_gather(
                        msgs[:, T:2 * T, :], table[cfg.half:, :], gi[:, 1, :],
                        T * P, cnt_regs[1], nhid, single_packet=False)
                    oh = opool.tile([P, 2 * T, P], GDT)
                    nc.vector.tensor_tensor(
                        out=oh[:],
                        in0=ld_t[:, b, :].to_broadcast([P, 2 * T, P]),
                        in1=iota_t[:].unsqueeze(1).to_broadcast([P, 2 * T, P]),
                        op=mybir.AluOpType.is_equal)
                    nc.vector.tensor_tensor(
                        out=oh[:], in0=oh[:],
                        in1=vals_t[:, b, :].to_broadcast([P, 2 * T, P]),
                        op=mybir.AluOpType.mult)
                    pm = pmpool.tile([P, nhid], F32, space="PSUM")
                    nc.tensor.matmul(
                        pm[:], ones_t[:], biasrow_t[:, l, :],
                        start=True, stop=False)
                    for t in range(2 * T):
                        nc.tensor.matmul(
                            pm[:], oh[:, t, :], msgs[:, t, :],
                            start=False, stop=(t == 2 * T - 1))
                    h_sb = hpool.tile([P, nhid], F32)
                    nc.scalar.activation(
                        h_sb[:], pm[:], mybir.ActivationFunctionType.Relu)
                    if l < N_LAYERS - 1:
                        nc.sync.dma_start(
                            h_dram[l][b * P:(b + 1) * P, :], h_sb[:])
                    else:
                        nc.sync.dma_start(
                            out_d[b * P:(b + 1) * P, :], h_sb[:])

    nc.compile()
    return nc


_CACHE = {}


def _assemble_output(outs, cfg, n_x, pos_idx):
    """Place per-core shard results into the padded output via pos_idx."""
    h_all = np.concatenate([o.astype(np.float32) for o in outs], axis=0)[:n_x]
    out = np.zeros((cfg.pad_n, cfg.nhid), np.float32)
    out[np.asarray(pos_idx).astype(np.int64)] = h_all
    return out


def kernel(**inputs):
    global LAST_RESULTS
    n_x = inputs["x"].shape[0]
    pos_idx = inputs["pos_idx"]
    cfg, in_maps = _host_prep(**inputs)
    k = cfg.key()
    if k not in _CACHE:
        _CACHE[k] = _build_program(cfg)
    nc = _CACHE[k]
    if os.environ.get("KERNEL_SIM"):
        from concourse.bass_interp import MultiCoreSim
        sim = MultiCoreSim(nc, num_cores=C, require_finite=True,
                           require_nnan=True)
        for c in range(C):
            cs = sim.cores[c]
            for name, arr in in_maps[c].items():
                cs.tensor(name)[:] = arr
            cs.tensor("out")[:] = 0.0
        sim.simulate(check_with_hw=False)
        outs = [np.array(sim.cores[c].tensor("out")) for c in range(C)]
        LAST_RESULTS = None
    else:
        res = None
        last_exc = None
        for attempt in range(3):
            try:
                res = run_bass_kernel_spmd(nc, in_maps, core_ids=list(range(C)))
                break
            except Exception as exc:  # flaky axon worker / wedged device
                last_exc = exc
                print(f"kernel: attempt {attempt} failed: {exc}", file=sys.stderr)
        if res is None:
            raise last_exc
        LAST_RESULTS = res
        outs = [res.results[c]["out"] for c in range(C)]
    return _assemble_output(outs, cfg, n_x, pos_idx)


# ---------------------------------------------------------------- self test
def _np_reference(x, motif_emb, adj_rows, adj_cols, adj_vals, pos_idx, pad_n,
                  w1, b1, w2, b2, w3, b3):
    h = np.concatenate([x, motif_emb], 0).astype(np.float64)
    n = h.shape[0]
    for w, b in ((w1, b1), (w2, b2), (w3, b3)):
        sup = h @ w.astype(np.float64)
        msgs = adj_vals[:, None].astype(np.float64) * sup[adj_cols]
        agg = np.zeros((n, w.shape[1]), np.float64)
        np.add.at(agg, adj_rows, msgs)
        h = np.maximum(agg + b, 0.0)
    h = h[: x.shape[0]]
    out = np.zeros((int(pad_n), h.shape[1]), np.float64)
    out[pos_idx] = h
    return out.astype(np.float32)


def _self_test(n_x=2800, n_motif=200, e=96000, nfeat=512, nhid=256, pad_n=4096,
               seed=0):
    rng = np.random.default_rng(seed)
    n = n_x + n_motif
    inputs = dict(
        x=rng.standard_normal((n_x, nfeat), dtype=np.float32),
        motif_emb=rng.standard_normal((n_motif, nfeat), dtype=np.float32),
        adj_rows=rng.integers(0, n, e),
        adj_cols=rng.integers(0, n, e),
        adj_vals=rng.random(e, dtype=np.float32),
        pos_idx=rng.permutation(pad_n)[:n_x],
        pad_n=np.int64(pad_n),
        w1=(rng.random((nfeat, nhid), dtype=np.float32) - 0.5) / np.sqrt(nhid),
        b1=(rng.random(nhid, dtype=np.float32) - 0.5) / np.sqrt(nhid),
        w2=(rng.random((nhid, nhid), dtype=np.float32) - 0.5) / np.sqrt(nhid),
        b2=(rng.random(nhid, dtype=np.float32) - 0.5) / np.sqrt(nhid),
        w3=(rng.random((nhid, nhid), dtype=np.float32) - 0.5) / np.sqrt(nhid),
        b3=(rng.random(nhid, dtype=np.float32) - 0.5) / np.sqrt(nhid),
    )
    expected = _np_reference(**inputs)
    got = kernel(**inputs)
    denom = np.abs(expected).max()
    err = np.abs(got - expected).max() / denom
    print(f"self-test abs-max rel err: {err:.3e}  (denom {denom:.3f})")
    assert err < 5e-3, "self test FAILED"
    print("SELF TEST PASS")


if __name__ == "__main__":
    _self_test()
